# revision 24
# baseline (speedup 1.0000x reference)
"""Trainium2 Bass kernel for nn_Attention_75299366633572.

Math (reference):
    scale[s] = temporal-PE flattened, s in [0, 1024)
    xs[n,s,:] = x[n,s,:] * scale[s]
    h = xs @ W.T + b                       # [N, S, 384]
    q,k,v = interleaved split of h         # each [N, S*128] via h[...,0::3] etc.
    scores = q @ k.T / sqrt(128)           # [128, 128]  (attention over batch!)
    out = softmax(scores) @ v              # [128, 131072]

Key algebraic restructure (per position s, with Wq' = Wq/sqrt(128)):
    scores[n,m] = sum_s xs_s[n,:] @ A @ xs_s[m,:].T  + (w . xs_s[m,:]) + rowconst
        A = Wq'.T @ Wk   [128,128],   w = Wk.T @ bq'  (bias term varying over m)
    row-constant terms (q_n.bk etc.) are softmax-invariant -> dropped.
    v bias: softmax rows sum to 1 -> out[n, (s,g)] += bv[g] added at the end.

Sharding: S (sequence) dim split across 8 cores (128 positions each).
Each core computes a partial [128,128] score matrix; partials are summed on
every core; softmax is replicated; each core emits its 16384 output cols.

v5: full fp16 datapath + remote-DMA score exchange.

fp16 keeps 10 mantissa bits (vs fp32r's 11); measured end-to-end rel-err
~6e-3 vs the 2e-2 gate. Wins vs the fp32r/fp32 baseline (141us):
  - input DMA halves (8.4MB -> 4.2MB), issued as 8 growing chunks (each
    dma_start costs ~700ns of queue issue time, so many small chunks
    serialize the load);
  - every matmul runs 1 cycle/row (fp32r at moving-dim 128 runs 4, fp32
    always 4) and fp16 weight loads take the FWL fast path;
  - the ncfw collective is replaced by a gpsimd remote_dma_broadcast:
    measured on this setup the collectives firmware does not wake until
    ~72us after kernel start no matter when the doorbell fires, putting a
    ~92us floor on any AllGather completion. remote_dma goes straight
    from Q7 SWDGE descriptors to the SDMA engines (SBUF->SBUF across the
    8 TPBs) and never touches the TOPSP firmware. Verified: all 64
    sender->receiver transfers land, slot = sender's partition id.

Exchange synchronization notes (hard-won):
  - rsem is bumped +2 by each of the 8 senders (16/8 per
    remote_dma_broadcast dest); the barrier is wait_ge(rsem, 16).
  - Do NOT clear the sems at kernel start: cross-core launch skew runs
    tens of us, so a late core's clear can wipe increments an early
    core's broadcast already delivered -> the barrier never reaches 16.
    Sems are cleared at kernel END instead (executions are host-paced,
    ms apart, so no skew hazard there), keeping re-execution safe.
  - The exchange lives in tc.tile_critical(): the Tile scheduler's
    timing sim cannot model remote increments and would deadlock on the
    wait; a critical section is scheduled as an opaque unit. Only gpsimd
    is barriered by it, so the V matmuls overlap the exchange — but the
    V PSUM pool must be created BEFORE the critical section (pools
    created after one get a sync dep on its post-crit boundary).
  - wait_critical_data_deps() splits the section: descriptor generation
    runs as soon as gpsimd is free (descriptors only hold addresses);
    the trigger waits for the partial scores to be in SBUF.

Per-core device pipeline:
  1. DMA XT = xs^T slice [128, 16384] f16 (ascending chunks).
  2. per 512-col chunk: YT = A^T @ XT_chunk (+w bias fused into the
     PSUM->SBUF cast, split DVE/ACT), then 4 accumulating score matmuls.
     Emission is software-pipelined (Y(c+1) queued before scores(c)).
  3. partial scores: remote_dma_broadcast to all 8 cores' SBUF,
     wait rsem >= 16.
  4. V_s = xs_s @ Wv^T per position, 4 positions per PSUM bank,
     [128,512] evacuation copies alternating DVE/ACT; overlaps the
     exchange.
  5. sum the 8 gathered partials (DVE tree), softmax, attn transpose.
  6. out = attnT^T @ V in [128,1024] chunks (two matmuls per 2-bank PSUM
     tile, one whole-chunk copy alternating DVE/ACT, *1/rowsum fused),
     DMA out on 3 rotating queues.
"""

import math

import numpy as np

import concourse.bass as bass
import concourse.mybir as mybir
import concourse.tile as tile
from concourse import bacc
from concourse.bass_utils import run_bass_kernel_spmd
from concourse.masks import make_identity

NCORES = 8
N = 128            # batch rows (attention is over this axis)
S = 1024           # sequence positions
D = 128            # feature dim
S_LOC = S // NCORES       # 128 positions per core
COLS = S_LOC * D          # 16384 free columns per core
F32 = mybir.dt.float32
F16 = mybir.dt.float16

_CACHE = {}


def _temporal_scale():
    """pe.flatten() from the reference's _temporal_pe, float32."""
    i = np.arange(32, dtype=np.float32)[:, None]
    j = np.arange(16, dtype=np.float32)[None, :]
    arg = (np.float32(1.0) * np.float32(np.pi) * i
           / np.power(np.float32(1000.0), (np.float32(2.0) * j / np.float32(128.0))))
    pe = np.stack([np.sin(arg), np.cos(arg)], axis=-1).reshape(32, 32)
    return pe.reshape(-1).astype(np.float32)   # [1024]


def _emit(nc, tc, xt_d, A_d, w_d, WvT_d, out_d):
    AX = mybir.AxisListType
    AF = mybir.ActivationFunctionType

    # Raw semaphores for the cross-core exchange. SPMD: every core runs this
    # same program, so the numbers agree across cores. Never released, so
    # Tile's own assigner (which draws from the same free pool) avoids them.
    rsem = nc.alloc_semaphore("rdma_recv")
    lsem = nc.alloc_semaphore("rdma_local")
    psem = nc.alloc_semaphore("rdma_prep")

    with (
        tc.tile_pool(name="consts", bufs=1) as consts,
        tc.tile_pool(name="xt", bufs=1) as xtp,
        tc.tile_pool(name="vbuf", bufs=1) as vp,
        tc.tile_pool(name="small", bufs=1) as small,
    ):
        ident = consts.tile([128, 128], F32)
        make_identity(nc, ident[:])
        A_sb = consts.tile([D, D], F16)
        nc.sync.dma_start(A_sb[:], A_d[:])
        WvT_sb = consts.tile([D, D], F16)
        w_sb = consts.tile([D, 1], F32)
        nc.sync.dma_start(w_sb[:], w_d[:])
        warm = consts.tile([128, 1], F32)

        XT = xtp.tile([128, COLS], F16)      # xs^T, [d, (s,n)]
        V = vp.tile([128, COLS], F16)        # v rows, [m, (s,g)]

        U32 = mybir.dt.uint32
        sc_part = small.tile([128, 128], F32, tag="scpart")
        allsc = small.tile([128, 8 * 128], F32, tag="allsc")
        flags = small.tile([128, NCORES], U32, tag="flags")
        flag_src = small.tile([128, 1], U32, tag="flagsrc")
        t512 = small.tile([128, 512], F32, tag="t512")
        t256 = small.tile([128, 256], F32, tag="t256")
        sc_full = small.tile([128, 128], F32, tag="scfull")
        ex = small.tile([128, 128], F32, tag="ex")
        attnT = small.tile([128, 128], F16, tag="attnT")
        mx = small.tile([128, 1], F32, tag="mx")
        sume = small.tile([128, 1], F32, tag="sume")
        rinv = small.tile([128, 1], F32, tag="rinv")

        # XT input: one HWDGE queue, ascending chunks that double in size —
        # small ones first so the first Y matmul starts after ~32 KiB, big
        # ones after so the per-dma_start issue cost doesn't serialize the
        # stream.
        bounds = [0, 128, 256, 512, 1024, 2048, 4096, 8192, 16384]
        for i, (lo, hi) in enumerate(zip(bounds[:-1], bounds[1:])):
            nc.sync.dma_start(XT[:, lo:hi], xt_d[:, lo:hi])
            if i == 3:
                nc.sync.dma_start(WvT_sb[:], WvT_d[:])

        # Warm the ACT function table during the DMA lead-in: the first
        # table-using ACTIVATE triggers a ~1.3us ACT_TABLE_LOAD, which
        # otherwise lands mid-phase-1 and stalls the first yt cast.
        nc.scalar.activation(warm[:], ident[:, 0:1], AF.Exp)

        # ---- Phase 1: Y = A^T @ XT (+w) and partial scores ----
        with (
            tc.tile_pool(name="yt", bufs=4) as ytp,
            tc.tile_pool(name="ps_y", bufs=3, space="PSUM") as ps_y,
            tc.tile_pool(name="ps_sc", bufs=1, space="PSUM") as ps_sc,
        ):
            sc_ps = ps_sc.tile([128, 128], F32)
            n_chunks = COLS // 512                # 32 chunks of 512 cols (4 s)
            yts = [None] * n_chunks

            def emit_y(c):
                yps = ps_y.tile([128, 512], F32, tag="y")
                nc.tensor.matmul(yps[:], A_sb[:], XT[:, c * 512:(c + 1) * 512],
                                 start=True, stop=True)
                yt = ytp.tile([128, 512], F16, tag="yt")
                # +w bias fused into the cast; split across DVE and ACT so
                # neither engine paces the chunk loop.
                nc.vector.tensor_scalar_add(yt[:, 0:256], yps[:, 0:256],
                                            w_sb[:, 0:1])
                nc.scalar.add(yt[:, 256:512], yps[:, 256:512], w_sb[:, 0:1])
                yts[c] = yt

            def emit_scores(c):
                yt = yts[c]
                for k in range(4):
                    s = 4 * c + k
                    nc.tensor.matmul(sc_ps[:], yt[:, k * 128:(k + 1) * 128],
                                     XT[:, s * 128:(s + 1) * 128],
                                     start=(s == 0), stop=(s == S_LOC - 1))

            emit_y(0)
            for c in range(1, n_chunks):
                emit_y(c)
                emit_scores(c - 1)
            emit_scores(n_chunks - 1)
            sc_done = nc.vector.tensor_copy(sc_part[:], sc_ps[:])

        # ps_v is created BEFORE the critical section so the V matmuls can
        # overlap the exchange (pools created after a critical section get
        # a sync dep on its post-crit boundary).
        with tc.tile_pool(name="ps_v", bufs=6, space="PSUM") as ps_v:
            # ---- Exchange partial scores: SBUF->SBUF remote DMA ----
            # Remote DATA writes work on this setup but remote SEMAPHORE
            # updates are silently dropped (measured: lsem reaches 16, rsem
            # only ever gets the self-loopback +2). So completion is
            # signalled with a second tiny remote write: after lsem==16
            # certifies my payload landed on every peer, I broadcast a
            # 1-word flag into slot <my id> of every peer's `flags` tile.
            # A receiver seeing flag[t] therefore knows payload[t] is fully
            # in its SBUF. Receivers spin-poll the 8 flag words on gpsimd.
            rdests = [(0, k) for k in range(NCORES)]
            with tc.tile_critical():
                nc.gpsimd.memset(flag_src[:], 1)
                rv = nc.gpsimd.partition_id()
                prep1 = nc.gpsimd.remote_dma_broadcast(
                    allsc[:, bass.ts(rv, 128)], sc_part[:],
                    remote_sem=rsem, local_sem=lsem, rdests=rdests)
                prep1.then_inc(psem, 1)
                prep2 = nc.gpsimd.remote_dma_broadcast(
                    flags[:, bass.ts(rv, 1)], flag_src[:],
                    remote_sem=rsem, local_sem=lsem, rdests=rdests)
                prep2.then_inc(psem, 1)
                nc.gpsimd.wait_ge(psem, 2)
                tc.wait_critical_data_deps()
                nc.gpsimd.trigger_dma(count=1)      # payload frame
                nc.gpsimd.wait_ge(lsem, 16)         # my payload delivered
                nc.gpsimd.trigger_dma(count=1)      # flag frame

                def _all_flags():
                    total = None
                    for t in range(NCORES):
                        r = nc.gpsimd.alloc_register(f"fpoll{t}")
                        nc.gpsimd.reg_load(r, flags[0:1, t:t + 1])
                        sv = nc.gpsimd.snap(r, donate=True)
                        total = sv if total is None else total + sv
                    return NCORES - total   # nonzero while any flag missing

                with nc.gpsimd.While(_all_flags):
                    nc.gpsimd.engine_nop()

            # ---- Phase 2: V projection (overlaps the exchange) ----
            # 4 positions share one PSUM bank -> one [128,512] copy.
            for g in range(S_LOC // 4):
                vps = ps_v.tile([128, 512], F32, tag="v")
                for k in range(4):
                    s = 4 * g + k
                    vm = nc.tensor.matmul(
                        vps[:, k * 128:(k + 1) * 128],
                        XT[:, s * 128:(s + 1) * 128], WvT_sb[:],
                        start=(k == 0), stop=(k == 3), skip_group_check=True)
                    if k == 0:
                        # pin each group behind the scores so the exchange
                        # fires ASAP (later k's chain via the bank WAW dep)
                        tile.add_dep_helper(vm.ins, sc_done.ins, sync=True,
                                            reason="V after scores")
                dst = V[:, g * 512:(g + 1) * 512]
                if g % 2 == 0:
                    nc.vector.tensor_copy(dst, vps[:])
                else:
                    nc.scalar.copy(dst, vps[:])

        # ---- sum the 8 gathered partials + softmax + attn transpose ----
        # (allsc is an output of the critical section, so the first add
        # waits for its post-crit boundary = rsem>=16 = all partials in.)
        nc.vector.tensor_add(t512[:], allsc[:, 0:512], allsc[:, 512:1024])
        nc.vector.tensor_add(t256[:], t512[:, 0:256], t512[:, 256:512])
        nc.vector.tensor_add(sc_full[:], t256[:, 0:128], t256[:, 128:256])
        nc.vector.reduce_max(out=mx[:], in_=sc_full[:], axis=AX.X, negate=True)
        nc.scalar.activation(ex[:], sc_full[:], AF.Exp,
                             bias=mx[:, 0:1], scale=1.0,
                             accum_out=sume[:, 0:1])
        nc.vector.reciprocal(rinv[:], sume[:])
        with tc.tile_pool(name="ps_at", bufs=1, space="PSUM") as ps_at:
            atps = ps_at.tile([128, 128], F32)
            nc.tensor.transpose(atps[:], ex[:], ident[:])
            nc.vector.tensor_copy(attnT[:], atps[:])

        # ---- Phase 3: out = attnT^T @ V, *rinv in the copy ----
        # [128,1024] chunks: 2 matmuls into a 2-bank PSUM tile, ONE
        # whole-chunk evacuation copy (alternating DVE/ACT), one 256 KiB
        # DMA per chunk on 3 rotating queues.
        with (
            tc.tile_pool(name="osb", bufs=4) as osbp,
            tc.tile_pool(name="ps_o", bufs=3, space="PSUM") as ps_o,
        ):
            for c in range(COLS // 1024):         # 16 chunks
                ops = ps_o.tile([128, 1024], F32, tag="o")
                nc.tensor.matmul(ops[:, 0:512], attnT[:],
                                 V[:, c * 1024:c * 1024 + 512],
                                 start=True, stop=True)
                nc.tensor.matmul(ops[:, 512:1024], attnT[:],
                                 V[:, c * 1024 + 512:(c + 1) * 1024],
                                 start=True, stop=True)
                osb = osbp.tile([128, 1024], F16, tag="osb")
                if c % 2 == 0:
                    nc.vector.tensor_scalar_mul(osb[:], ops[:], rinv[:, 0:1])
                else:
                    nc.scalar.mul(osb[:], ops[:], rinv[:, 0:1])
                eng = [nc.sync, nc.scalar, nc.gpsimd][c % 3]
                eng.dma_start(out_d[:, c * 1024:(c + 1) * 1024], osb[:])

        # Reset the exchange state for any later execution of this NEFF.
        # Safe here: the flag barrier passed, so every sender is done and
        # nothing is in flight; executions are host-paced, ms apart.
        nc.gpsimd.memset(flags[:], 0)
        nc.gpsimd.sem_clear(rsem)
        nc.gpsimd.sem_clear(lsem)
        nc.gpsimd.sem_clear(psem)


def _build():
    key = "v5"
    if key in _CACHE:
        return _CACHE[key]
    nc = bacc.Bacc("TRN2", target_bir_lowering=False, debug=False,
                   num_devices=NCORES)
    xt_d = nc.dram_tensor("xt", [128, COLS], F16, kind="ExternalInput")
    A_d = nc.dram_tensor("A", [D, D], F16, kind="ExternalInput")
    w_d = nc.dram_tensor("w", [D, 1], F32, kind="ExternalInput")
    WvT_d = nc.dram_tensor("WvT", [D, D], F16, kind="ExternalInput")
    out_d = nc.dram_tensor("out", [N, COLS], F16, kind="ExternalOutput")
    with tile.TileContext(nc) as tc:
        _emit(nc, tc, xt_d, A_d, w_d, WvT_d, out_d)
    nc.compile()
    _CACHE[key] = nc
    return nc


def prepare_inputs(x, W, b):
    """Host-side prep: shard + transpose x over S, build derived matrices."""
    x = np.asarray(x, dtype=np.float32)
    W = np.asarray(W, dtype=np.float32)
    b = np.asarray(b, dtype=np.float32)

    rs = math.sqrt(float(D))
    Wq = W[0::3, :].astype(np.float64) / rs
    Wk = W[1::3, :].astype(np.float64)
    Wv = W[2::3, :]
    bq = b[0::3].astype(np.float64) / rs
    bv = b[2::3]

    A = (Wq.T @ Wk).astype(np.float16)                       # [128, 128]
    w = (Wk.T @ bq).astype(np.float32)[:, None]              # [128, 1]
    WvT = np.ascontiguousarray(Wv.T).astype(np.float16)      # [128, 128]

    scale = _temporal_scale()                                # [1024]
    in_maps = []
    for c in range(NCORES):
        sl = slice(c * S_LOC, (c + 1) * S_LOC)
        xs_c = x[:, sl, :] * scale[sl][None, :, None]        # [n, s, d] f32
        xt_c = np.ascontiguousarray(
            xs_c.transpose(2, 1, 0)).reshape(D, COLS).astype(np.float16)
        in_maps.append({
            "xt": xt_c, "A": A, "w": w, "WvT": WvT,
        })
    return in_maps, bv


def run(inputs, trace=False, **kw):
    nc = _build()
    in_maps, bv = prepare_inputs(inputs["x"], inputs["W"], inputs["b"])
    res = run_bass_kernel_spmd(nc, in_maps, core_ids=list(range(NCORES)),
                               trace=trace, **kw)
    out = np.concatenate(
        [res.results[c]["out"].astype(np.float32) for c in range(NCORES)], axis=1)
    out += np.tile(bv, S)[None, :]     # v-bias: attn rows sum to 1
    return out, res


def kernel(x, W, b):
    out, _ = run({"x": x, "W": W, "b": b})
    return out
